# revision 26
# baseline (speedup 1.0000x reference)
"""Trainium2 Bass kernel for nn_BasicTransformerLayer (dense transformer layer).

Strategy:
- Data-parallel over batch: B=8, one batch element per NeuronCore, no
  collectives (compute-bound regime; per-core work is identical).
- Activations transposed [features, tokens]; residual stream in fp32r.
- fp8e4m3 DoubleRow matmuls (2 contraction k-tiles per MM) for every
  attention-path projection (Q/K/V/O, both attns) and for P@V; the FFN
  stays f16 (fp8 there costs ~5e-2 rel err — it feeds the output with no
  residual damping). Scales (all powers of 2): activations x16, weights
  x256; descale 2^-12 folds into the psum-read op. The V tiles carry x16
  so AT=PV/sums lands pre-scaled for the fp8 O-proj with no extra op.
- Pair-tile layouts for DoubleRow 3D APs: LN outputs, context, V and AT
  live as [128, 2*W] tiles (k-tile pairs side by side in the free dim).
- Softmax: scores for pairs of kv tiles land in one [128,1024] 2-bank psum
  so exp amortizes the scalar engine's ACTIVATE overhead; the
  multiplicative exp(bias)*16 (host-precomputed f16) applies on vector in
  f16 (2x DVE) casting to the fp8 pe used by the DoubleRow P@V; the kv-sum
  (softmax denominator) is folded into the P@V matmul via V tiles laid out
  [64 ones | 64 values] per head: psum rows 0:64 = sums, 64:128 = P@V.
- PSUM: 3 double-bank score tiles + 2 P@V banks; BLK=3 pair-blocks keep the
  score-psum WAR distance >= the exp latency so the PE never burst-stalls.
- Q-projections run as fillers inside the previous attention loop's idle
  slots; fn-layernorm runs as post_qc of self-attn qc0 to overlap qc1.
- exp(bias) streams on the gpsimd DMA ring only; xT splits across the
  scalar+vector rings so LN1 starts ~7us in; output written per-psum-bank
  with rotating buffers over 3 DMA rings, copies alternating DVE/ACT.
"""
import sys

sys.path.insert(0, '/opt/trn_rl_repo')

import numpy as np

E, C, H, D, FF = 768, 512, 12, 64, 3072
B, S, L = 8, 1024, 256
EPS = 1e-5
NCORES = 8
QCH = 512                  # q-chunk (matmul moving free dim)
NQ = S // QCH              # 2
JE = E // 128              # 6 feature tiles
JC = C // 128              # 4 cross-feature tiles
JF = FF // 128             # 24 ffn tiles
KVS = S // 128             # 8 self kv tiles
KVC = L // 128             # 2 cross kv tiles

SX = 16.0                  # fp8 activation scale
SW = 256.0                 # fp8 weight scale
DS = 1.0 / (SX * SW)       # psum descale 2^-12

_BUILT = {}
TRACE = False
LAST = {}
PHASES = []


def _build(flags):
    import concourse.bacc as bacc
    import concourse.mybir as mybir
    import concourse.tile as tile
    from concourse.tile import add_dep_helper
    from concourse.bass import AP as BassAP

    FR = mybir.dt.float32r
    F32 = mybir.dt.float32
    F16 = mybir.dt.float16
    F8 = mybir.dt.float8e4
    AF = mybir.ActivationFunctionType
    OP = mybir.AluOpType
    DRM = mybir.MatmulPerfMode.DoubleRow

    nc = bacc.Bacc("TRN2", target_bir_lowering=False, debug=False,
                   enable_asserts=True, num_devices=NCORES)

    def din(name, shape, dt=F16):
        return nc.dram_tensor(name, shape, dt, kind="ExternalInput").ap()

    xT_d = din("xT", [E, S], F32)
    ctxT_d = din("ctxT", [C, L], F8)
    # pre-tiled fp8 weights: [ofn, 128, jin*128] (contiguous per of)
    w_d = {
        'wq_c': din("wq_c", [JE, 128, JE * 128], F8),
        'wk_c': din("wk_c", [JE, 128, JC * 128], F8),
        'wo_c': din("wo_c", [JE, 128, JE * 128], F8),
        'wq_s': din("wq_s", [JE, 128, JE * 128], F8),
        'wk_s': din("wk_s", [JE, 128, JE * 128], F8),
        'wo_s': din("wo_s", [JE, 128, JE * 128], F8),
        'w1': din("w1", [JF, 128, JE * 128]),
        'w2': din("w2", [JF, 128, JE * 128]),
        # V weights: [jin, 128, E] (row blocks of original)
        'wv_c': din("wv_c", [JC, 128, E], F8),
        'wv_s': din("wv_s", [JE, 128, E], F8),
    }
    expb_c_d = din("expb_c", [H, L, S])
    expb_s_d = din("expb_s", [H, S, S])
    VIDX = {}
    _off = 0
    for nm, ln in [('cn_g', JE), ('cn_b', JE), ('sn_g', JE), ('sn_b', JE),
                   ('fn_g', JE), ('fn_b', JE), ('bq_c', JE), ('bk_c', JE),
                   ('bo_c', JE), ('bq_s', JE), ('bk_s', JE), ('bo_s', JE),
                   ('b1', JF), ('b2', JE)]:
        VIDX[nm] = _off
        _off += ln
    NV = _off
    vecs_d = din("vecs", [128, NV], F32)
    yT_d = nc.dram_tensor("yT", [E, S], F32, kind="ExternalOutput").ap()

    with tile.TileContext(nc) as tc:
        with tc.tile_pool(name="const", bufs=1) as cpool, \
             tc.tile_pool(name="acts", bufs=1) as acts, \
             tc.tile_pool(name="wst", bufs=8) as wst, \
             tc.tile_pool(name="tr", bufs=2) as tr, \
             tc.tile_pool(name="pe", bufs=4) as pepool, \
             tc.tile_pool(name="eb", bufs=4) as ebpool, \
             tc.tile_pool(name="ps", bufs=1, space="PSUM") as ps:

            def T(pool, shape, dtype, tag, bufs=1):
                return pool.tile(shape, dtype, tag=tag, name=tag, bufs=bufs)

            ones = T(cpool, [128, 128], FR, "ones")
            ones16 = T(cpool, [128, 128], F16, "ones16")
            ones_f = T(cpool, [128, 128], F32, "ones_f")
            epsc = T(cpool, [128, 1], F32, "epsc")
            epsc8 = T(cpool, [128, 1], F32, "epsc8")
            nc.vector.memset(epsc[:], EPS)
            nc.vector.memset(epsc8[:], EPS / (SX * SX))
            nc.vector.memset(ones_f[:], 1.0)
            nc.vector.tensor_copy(ones[:], ones_f[:])
            nc.vector.tensor_copy(ones16[:], ones_f[:])
            vecs = T(cpool, [128, NV], F32, "vecs")
            # gpsimd ring: its first eb load is far away, so vecs lands early
            nc.gpsimd.dma_start(vecs[:], vecs_d[:])

            def vap(nm, j):
                return vecs[:, VIDX[nm] + j:VIDX[nm] + j + 1]

            # persistent activation tiles
            rA = [T(acts, [128, S], FR, f"rA{j}") for j in range(JE)]
            rB = [T(acts, [128, S], FR, f"rB{j}") for j in range(JE)]
            # fp8 LN outputs as pair tiles (k-tile pairs side by side)
            lnP = [T(acts, [128, 2 * S], F8, f"lnP{p}") for p in range(JE // 2)]
            # f16 LN output for the FFN path
            lnF = [T(acts, [128, S], F16, f"lnF{j}") for j in range(JE)]
            KT = [T(acts, [128, S], F16, f"KT{j}") for j in range(JE)]
            # V pair tiles: per kv-tile-pair [128, 2*H*128] fp8;
            # within each half, head h cols = [64 ones | 64 values]
            VP = [T(acts, [128, 2 * H * 128], F8, f"VP{p}")
                  for p in range(KVS // 2)]
            QT = [T(acts, [128, QCH], F16, f"QT{j}") for j in range(JE)]
            QTb = [T(acts, [128, QCH], F16, f"QU{j}") for j in range(JE)]
            # fp8 attention output as pair tiles
            AT8 = [T(acts, [128, 2 * QCH], F8, f"AT{p}") for p in range(JE // 2)]
            ctxP = [T(acts, [128, 2 * L], F8, f"cx{p}") for p in range(JC // 2)]

            def pair3(tile_, two_stride, n, off=0):
                """3D AP [[p,128],[two_stride,2],[1,n]] at free offset off."""
                vp = tile_[:]
                return BassAP(vp.tensor, vp.offset + off,
                              [[vp.ap[0][0], 128], [two_stride, 2], [1, n]])

            def vp_strided(t, head0, nh, ones_cols):
                vp = VP[t // 2][:]
                pstride = vp.ap[0][0]
                off = (vp.offset + (t % 2) * H * 128 + head0 * 128
                       + (0 if ones_cols else 64))
                return BassAP(vp.tensor, off,
                              [[pstride, 128], [128, nh], [1, 64]])

            for t in range(KVS):
                nc.vector.memset(vp_strided(t, 0, H, True), 1.0)

            for p in range(JC // 2):
                nc.sync.dma_start(ctxP[p][:, 0:L],
                                  ctxT_d[(2 * p) * 128:(2 * p + 1) * 128, :])
                nc.sync.dma_start(ctxP[p][:, L:2 * L],
                                  ctxT_d[(2 * p + 1) * 128:(2 * p + 2) * 128, :])

            def load_xT():
                # scalar ring is idle at start: the whole first q-chunk
                # streams there in parallel with the cross-K/V weight DMAs
                # on the sync ring, so ln1 qc0 starts ~7us in
                for half in range(2):
                    cs_ = slice(half * QCH, (half + 1) * QCH)
                    for j in range(JE):
                        ring = nc.scalar if half == 0 else \
                            (nc.sync if j % 2 == 0 else nc.scalar)
                        ring.dma_start(
                            rA[j][:, cs_],
                            xT_d[j * 128:(j + 1) * 128, cs_].bitcast(FR))

            # PSUM: 3x double-bank "scp" tiles (scores/general) + 2 single
            # "pv" banks = 8 banks total.
            def psum_tile(tag, n=QCH):
                return ps.tile([128, n], F32, tag=tag, name=tag)

            def scp_tile(k):
                return psum_tile(f"scp{k % 3}", 2 * QCH)

            _rot = {'i': 0, 'cur': None}

            def rot_ps(n=QCH):
                i = _rot['i']
                _rot['i'] += 1
                if i % 2 == 0:
                    _rot['cur'] = scp_tile(i // 2)
                return _rot['cur'][:, (i % 2) * QCH:(i % 2) * QCH + n]

            # ---------------- layer norm (transposed layout) ----------------
            # fp8 mode: output = 16*(x-mu)*rstd into lnP pair tiles (the x16
            # rides in rstd via the ln-scale); f16 mode: plain into lnF.
            # rstd = exp(-0.5*ln((var+eps)/s)) keeps ACT on the exp table set
            # (no Sqrt table thrash); x^2 tiles ride on gpsimd.
            def ln_phase(src, gname, bname, affine, fp8, only_qc=None,
                         gps_apply=False):
                inv = 1.0 / float(E)
                for qc in range(NQ):
                    if only_qc is not None and qc != only_qc:
                        continue
                    qs = slice(qc * QCH, (qc + 1) * QCH)
                    sqs = []
                    for j in range(JE):
                        sq = T(tr, [128, QCH], F16, "sq", bufs=3)
                        nc.gpsimd.tensor_tensor(sq[:], src[j][:, qs],
                                                src[j][:, qs], op=OP.mult)
                        sqs.append(sq)
                    s12 = scp_tile(0)
                    s1 = s12[:, 0:QCH]
                    s2 = s12[:, QCH:2 * QCH]
                    for j in range(JE):
                        nc.tensor.matmul(s1, ones[:, 0:128], src[j][:, qs],
                                         start=(j == 0), stop=(j == JE - 1))
                    for j in range(JE):
                        nc.tensor.matmul(s2, ones16[:, 0:128], sqs[j][:],
                                         start=(j == 0), stop=(j == JE - 1))
                    m1 = T(tr, [128, QCH], F32, "m1m", bufs=1)
                    nc.vector.tensor_scalar(m1[:], s1[:], inv, None,
                                            op0=OP.mult)
                    t1 = T(tr, [128, QCH], F32, "t1m", bufs=1)
                    nc.gpsimd.tensor_tensor(t1[:], m1[:], m1[:], op=OP.mult)
                    var = T(tr, [128, QCH], F32, "var", bufs=1)
                    nc.vector.scalar_tensor_tensor(var[:], s2[:], inv, t1[:],
                                                   op0=OP.mult, op1=OP.subtract)
                    sc_ = (1.0 / (SX * SX)) if fp8 else 1.0
                    nc.scalar.activation(var[:], var[:], AF.Ln,
                                         bias=(epsc8 if fp8 else epsc)[:, 0:1],
                                         scale=sc_)
                    rstd = T(tr, [128, QCH], F32, "rstd", bufs=1)
                    nc.scalar.activation(rstd[:], var[:], AF.Exp, scale=-0.5)
                    m1r = T(tr, [128, QCH], F16, "m1r", bufs=1)
                    nc.vector.tensor_tensor(m1r[:], m1[:], rstd[:], op=OP.mult)
                    for j in range(JE):
                        if fp8:
                            dst = lnP[j // 2][:, (j % 2) * S + qc * QCH:
                                              (j % 2) * S + (qc + 1) * QCH]
                        else:
                            dst = lnF[j][:, qs]
                        tmp = T(tr, [128, QCH], F16, "lntmp", bufs=2)
                        eng = nc.gpsimd if (gps_apply and j % 2) else nc.vector
                        eng.tensor_tensor(tmp[:], src[j][:, qs], rstd[:],
                                          op=OP.mult)
                        if affine:
                            tmp2 = T(tr, [128, QCH], F16, "lntmp2", bufs=2)
                            nc.vector.tensor_tensor(tmp2[:], tmp[:], m1r[:],
                                                    op=OP.subtract)
                            nc.vector.tensor_scalar(dst, tmp2[:],
                                                    vap(gname, j), vap(bname, j),
                                                    op0=OP.mult, op1=OP.add)
                        else:
                            nc.vector.tensor_tensor(dst, tmp[:],
                                                    m1r[:], op=OP.subtract)

            # -------- fp8 DoubleRow projection from pre-tiled weights --------
            def wload(wd, of, jin, ring=None):
                wt = T(wst, [128, JE * 128], F8, "wg", bufs=4)
                (ring or nc.sync).dma_start(wt[:, 0:jin * 128], wd[of])
                return wt

            _cpn = {'i': 0}

            def psum_read(out_ap, pt, bias_ap):
                # out = pt * 2^-12 (+ bias); gpsimd can't read PSUM, so
                # rotate 2:1 vector:scalar (ACT carries the exp stream)
                if bias_ap is not None:
                    nc.vector.tensor_scalar(out_ap, pt, DS, bias_ap,
                                            op0=OP.mult, op1=OP.add)
                elif _cpn['i'] % 2 == 1:
                    _cpn['i'] += 1
                    nc.scalar.mul(out_ap, pt, DS)
                else:
                    _cpn['i'] += 1
                    nc.vector.tensor_scalar(out_ap, pt, DS, None, op0=OP.mult)

            def wchain(wt, jin, src3, out_ap, n, bias_ap, pt=None):
                """src3(p) -> 3D rhs AP for k-tile pair p."""
                if pt is None:
                    pt = rot_ps(n)
                wv_ = wt[:]
                for p in range(jin // 2):
                    w3 = BassAP(wv_.tensor, wv_.offset + p * 256,
                                [[wv_.ap[0][0], 128], [128, 2], [1, 128]])
                    nc.tensor.matmul(pt, w3, src3(p),
                                     start=(p == 0), stop=(p == jin // 2 - 1),
                                     perf_mode=DRM)
                psum_read(out_ap, pt, bias_ap)

            def ln_src3(qs0):
                return lambda p: pair3(lnP[p], S, QCH, off=qs0)

            def qproj_closures(prefix, wq, qs0, qt_set, pslot=None):
                outs = []
                for of in range(JE):
                    def one(of=of):
                        wt = wload(wq, of, JE)
                        wchain(wt, JE, ln_src3(qs0),
                               qt_set[of][:], QCH,
                               vap(f'bq_{prefix}', of)
                               if flags[f'bq_{prefix}'] else None,
                               pt=pslot() if pslot else None)
                    outs.append(one)
                return outs

            _frot = {'i': 0, 'cur': None}

            def fill_ps(n=QCH):
                i = _frot['i']
                _frot['i'] += 1
                if i % 2 == 0:
                    _frot['cur'] = psum_tile("scp2", 2 * QCH)
                return _frot['cur'][:, (i % 2) * QCH:(i % 2) * QCH + n]

            # ---------------- K/V projection emission ----------------
            def emit_kv(prefix, kv_pairs, kv_stride, wk, wv, jin_kv, kv_len):
                nkv = kv_len // 128

                _kps = {'i': 0}

                def emit_k(of):
                    wt = wload(wk, of, jin_kv)
                    for ks in range(0, kv_len, QCH):
                        n = min(QCH, kv_len - ks)
                        kp = psum_tile(f"pv{_kps['i'] % 2}")
                        _kps['i'] += 1
                        wchain(wt, jin_kv,
                               lambda p: pair3(kv_pairs[p], kv_stride, n,
                                               off=ks),
                               KT[of][:, ks:ks + n], n,
                               vap(f'bk_{prefix}', of)
                               if flags[f'bk_{prefix}'] else None,
                               pt=kp[:, 0:n])

                def emit_vgroup(os_, tg):
                    n = min(QCH, E - os_)
                    tcnt = min(4, nkv - tg)
                    vts = [scp_tile(1), scp_tile(2)]
                    vps = [vts[i // 2][:, (i % 2) * QCH:(i % 2) * QCH + n]
                           for i in range(tcnt)]
                    for jp in range(jin_kv // 2):
                        wt = T(wst, [128, 2 * QCH], F8, "wv", bufs=4)
                        nc.sync.dma_start(wt[:, 0:n],
                                          wv[2 * jp, :, os_:os_ + n])
                        nc.sync.dma_start(wt[:, n:2 * n],
                                          wv[2 * jp + 1, :, os_:os_ + n])
                        for i in range(tcnt):
                            nc.tensor.matmul(
                                vps[i][:, 0:n],
                                pair3(kv_pairs[jp], kv_stride, 128,
                                      off=(tg + i) * 128),
                                pair3(wt, n, n),
                                start=(jp == 0),
                                stop=(jp == jin_kv // 2 - 1),
                                perf_mode=DRM)
                    for i in range(tcnt):
                        dst = vp_strided(tg + i, os_ // 64, n // 64, False)
                        src = vps[i][:, 0:n].rearrange("p (h d) -> p h d", d=64)
                        # V carries x16: psum(4096) * 2^-8 = 16*V
                        nc.scalar.mul(dst, src, DS * SX)

                vgroups = [(os_, tg) for os_ in range(0, E, QCH)
                           for tg in range(0, nkv, 4)]
                for i in range(max(JE, len(vgroups))):
                    if i < JE:
                        emit_k(i)
                    if i < len(vgroups):
                        emit_vgroup(*vgroups[i])

            # ---------------- attention (shared cross/self) ----------------
            def attention(prefix, kv_pairs, kv_stride, expb_d, res_in,
                          res_out, wq, wk, wv, wo, jin_kv, kv_len,
                          post_qc=None, kv_done=False, qt_sets=None,
                          emit_q=(True, True), fillers=None, rot_mods=(3, 3)):
                if not kv_done:
                    emit_kv(prefix, kv_pairs, kv_stride, wk, wv, jin_kv,
                            kv_len)
                if qt_sets is None:
                    qt_sets = [QT, QT]

                for qc in range(NQ):
                    PHASES.append((f'{prefix}:qc{qc}',
                                   int(__import__('re').findall(
                                       r'\d+', nc.get_next_instruction_name())[0])))
                    rot_mod = rot_mods[qc]
                    qs = slice(qc * QCH, (qc + 1) * QCH)
                    qt = qt_sets[qc]
                    # Q^T for this q-chunk (scale folded into wq on host)
                    if emit_q[qc]:
                        for fn_ in qproj_closures(prefix, wq, qc * QCH, qt):
                            fn_()
                    fq = list(fillers[qc]) if fillers else []
                    nkt = kv_len // 128
                    npair = nkt // 2
                    seq = [(h, kp) for h in range(H) for kp in range(npair)]
                    _sr = {'i': 0}
                    state = {}

                    def load_eb(h):
                        ebts = []
                        for kp in range(npair):
                            ebt = T(ebpool, [128, 2 * QCH], F16, "eb", bufs=8)
                            # split the eb stream across two DMA rings
                            ring = nc.gpsimd if (h + kp) % 2 else nc.sync
                            ring.dma_start(
                                ebt[:].rearrange("p (t c) -> p t c", t=2),
                                expb_d[h, kp * 256:(kp + 1) * 256, qs]
                                .rearrange("(t p) c -> p t c", p=128))
                            ebts.append(ebt)
                        state.setdefault(h, {'tiles': []})['ebts'] = ebts

                    load_eb(0)

                    def s_stage(i):
                        h, kp = seq[i]
                        st = state.setdefault(h, {'tiles': []})
                        if kp == 0 and h + 1 < H:
                            load_eb(h + 1)
                        th, ph = (h * D) // 128, (h * D) % 128
                        sc = scp_tile(_sr['i'] % rot_mod)
                        _sr['i'] += 1
                        for half in range(2):
                            kvt = 2 * kp + half
                            chain(nc.tensor.matmul(
                                sc[:, half * QCH:(half + 1) * QCH],
                                KT[th][ph:ph + D, kvt * 128:(kvt + 1) * 128],
                                qt[th][ph:ph + D, :], start=True, stop=True))
                        pe = T(pepool, [128, 2 * QCH], F16, "pe", bufs=4)
                        pe8 = T(pepool, [128, 2 * QCH], F8, "pe8", bufs=6)
                        nc.scalar.activation(pe[:], sc[:], AF.Exp)
                        # half the exp(bias) multiplies ride on gpsimd (DVE
                        # is the bottleneck across the attention windows)
                        gps = (kp >= 2) if npair > 1 else (h % 2 == 1)
                        eng = nc.gpsimd if gps else nc.vector
                        eng.tensor_tensor(pe8[:], pe[:],
                                          st['ebts'][kp][:], op=OP.mult)
                        st['tiles'].append(pe8)

                    def pv_stage(i):
                        h, kp = seq[i]
                        st = state[h]
                        th, ph = (h * D) // 128, (h * D) % 128
                        if kp == 0:
                            st['pv'] = psum_tile(f"pv{h % 2}")
                        pe8 = st['tiles'][kp]
                        vp = VP[kp][:]
                        lhs3 = BassAP(vp.tensor, vp.offset + h * 128,
                                      [[vp.ap[0][0], 128], [H * 128, 2],
                                       [1, 128]])
                        chain(nc.tensor.matmul(
                            st['pv'][:], lhs3, pair3(pe8, QCH, QCH),
                            start=(kp == 0), stop=(kp == npair - 1),
                            perf_mode=DRM))
                        if kp == npair - 1:
                            pv = st['pv']

                            def fin(pv=pv, th=th, ph=ph, h=h):
                                # psum rows 0:64 = kv-sums, 64:128 = P@V(x16)
                                rec = T(tr, [64, QCH], F32, "rec", bufs=2)
                                nc.vector.reciprocal_approx_fast(
                                    rec[:], pv[0:64, :])
                                dst = AT8[th // 2][ph:ph + D,
                                                  (th % 2) * QCH:
                                                  (th % 2 + 1) * QCH]
                                nc.vector.tensor_tensor(dst, pv[64:128, :],
                                                        rec[:], op=OP.mult)
                            deferred.append(fin)
                            del state[h]

                    _pe_chain = {'prev': None}

                    def chain(bi):
                        if _pe_chain['prev'] is not None:
                            add_dep_helper(bi.ins, _pe_chain['prev'].ins,
                                           sync=False, reason="pe-order")
                        _pe_chain['prev'] = bi

                    deferred = []
                    BLK = 3
                    blocks = [list(range(i, min(i + BLK, len(seq))))
                              for i in range(0, len(seq), BLK)]
                    for j in range(len(blocks) + 1):
                        if j < len(blocks):
                            for i in blocks[j]:
                                s_stage(i)
                        if fq and j >= 1:
                            # pace the backlog evenly over remaining blocks
                            npop = -(-len(fq) // max(1, len(blocks) + 1 - j))
                            for _ in range(min(npop, len(fq))):
                                fq.pop(0)()
                        while len(deferred) > 1:
                            deferred.pop(0)()
                        if j >= 1:
                            for i in blocks[j - 1]:
                                pv_stage(i)
                    while deferred:
                        deferred.pop(0)()
                    for fn_ in fq:
                        fn_()
                    # out-projection + residual (AT carries x16, wo x256)
                    at3 = lambda p: pair3(AT8[p], QCH, QCH)
                    for of in range(JE):
                        wt = wload(wo, of, JE)
                        pt = rot_ps()
                        wv_ = wt[:]
                        for p in range(JE // 2):
                            w3 = BassAP(wv_.tensor, wv_.offset + p * 256,
                                        [[wv_.ap[0][0], 128], [128, 2],
                                         [1, 128]])
                            nc.tensor.matmul(pt, w3, at3(p),
                                             start=(p == 0),
                                             stop=(p == JE // 2 - 1),
                                             perf_mode=DRM)
                        if flags[f'bo_{prefix}']:
                            tmp = T(tr, [128, QCH], F32, "obias", bufs=2)
                            nc.vector.tensor_scalar(tmp[:], pt, DS,
                                                    vap(f'bo_{prefix}', of),
                                                    op0=OP.mult, op1=OP.add)
                            nc.vector.tensor_tensor(res_out[of][:, qs], tmp[:],
                                                    res_in[of][:, qs],
                                                    op=OP.add)
                        else:
                            nc.vector.scalar_tensor_tensor(
                                res_out[of][:, qs], pt, DS,
                                res_in[of][:, qs], op0=OP.mult, op1=OP.add)
                    if post_qc is not None:
                        post_qc(qc)

            # ================= the layer =================
            import re as _re

            def _mark(lbl):
                n = int(_re.findall(r'\d+', nc.get_next_instruction_name())[0])
                PHASES.append((lbl, n))

            _mark('ckv')
            # cross K/V only need ctxP: emit first to overlap xT DMA
            emit_kv('c', ctxP, L, w_d['wk_c'], w_d['wv_c'], JC, L)
            load_xT()
            _mark('ln1')
            ln_phase(rA, 'cn_g', 'cn_b', flags['cn'], fp8=True)
            _mark('cross')
            fill_q0 = qproj_closures('c', w_d['wq_c'], QCH, QTb,
                                     pslot=fill_ps)
            fill_q1 = qproj_closures('s', w_d['wq_s'], 0, QT,
                                     pslot=fill_ps)
            attention('c', ctxP, L, expb_c_d, rA, rB,
                      w_d['wq_c'], w_d['wk_c'], w_d['wv_c'], w_d['wo_c'],
                      JC, L, kv_done=True, rot_mods=(2, 2),
                      qt_sets=[QT, QTb], emit_q=(True, False),
                      fillers=[fill_q0, fill_q1],
                      post_qc=lambda qc: ln_phase(rB, 'sn_g', 'sn_b',
                                                  flags['sn'], fp8=True,
                                                  only_qc=qc,
                                                  gps_apply=True))
            # ---- FFN first-matmul emission (shared by filler + main) ----
            # Writes pre-gelu f16 tiles; the gelu itself batches into the
            # FFN phases (Exp and Gelu can NEVER share an ACT table set, so
            # a gelu inside the attention window would thrash tables; the
            # psum-read here is a Copy, which lives in every set).
            def emit_f1(qc, of, f1_ap):
                qs_ = slice(qc * QCH, (qc + 1) * QCH)
                wt = T(wst, [128, JE * 128], F16, "w1g", bufs=3)
                nc.sync.dma_start(wt[:], w_d['w1'][of])
                for j in range(JE):
                    nc.tensor.matmul(f1_ap, wt[:, j * 128:(j + 1) * 128],
                                     lnF[j][:, qs_],
                                     start=(j == 0), stop=(j == JE - 1),
                                     skip_group_check=True)
                g = T(acts, [128, QCH], F16, "g", bufs=26)
                nc.scalar.copy(g[:], f1_ap)
                return g

            g0 = []
            _ft = {}

            def f1_filler_closures():
                # f1(qc0) of-tiles run as fillers inside self-attn qc1,
                # double-buffered in the two halves of ONE scp2 allocation
                # (lazy: scores in qc0 still rotate through scp2)
                outs = []
                for of in range(JF):
                    def one(of=of):
                        if 'ft' not in _ft:
                            _ft['ft'] = scp_tile(2)
                        f1 = _ft['ft']
                        g0.append(emit_f1(0, of,
                                          f1[:, (of % 2) * QCH:
                                             (of % 2 + 1) * QCH]))
                    outs.append(one)
                return outs

            _mark('self')
            # fn-ln per qc as post_qc: qc0's fn-ln + f1(qc0) overlap self qc1
            attention('s', lnP, S, expb_s_d, rB, rA,
                      w_d['wq_s'], w_d['wk_s'], w_d['wv_s'], w_d['wo_s'],
                      JE, S, qt_sets=[QT, QTb], emit_q=(False, True),
                      rot_mods=(3, 2),
                      fillers=[[], f1_filler_closures()],
                      post_qc=lambda qc: ln_phase(rA, 'fn_g', 'fn_b',
                                                  flags['fn'], fp8=False,
                                                  only_qc=qc))
            _mark('ffn')

            # ================= FFN second matmuls (f16) =================
            # phase B: w2 chain for qc0 (g0 precomputed) interleaved with
            # f1(qc1); phase C: w2 chain for qc1.
            def load_w2(of):
                w2t = T(wst, [128, JE * 128], F16, "w2g", bufs=3)
                nc.gpsimd.dma_start(w2t[:], w_d['w2'][of])
                return w2t

            g1 = []
            # w2 weight prefetch runs as one continuous stream across both
            # qc phases so the B->C transition never starves LDWEIGHTS
            w2q = [load_w2(0), load_w2(1)]
            _w2n = {'i': 2}

            def next_w2():
                if _w2n['i'] < 2 * JF:
                    w2q.append(load_w2(_w2n['i'] % JF))
                    _w2n['i'] += 1
                return w2q.pop(0)

            for qc in range(NQ):
                qs = slice(qc * QCH, (qc + 1) * QCH)
                ytiles = [scp_tile(k) for k in range(3)]
                ypt = [ytiles[k // 2][:, (k % 2) * QCH:(k % 2 + 1) * QCH]
                       for k in range(JE)]
                gsrc = g0 if qc == 0 else g1
                for of in range(JF):
                    if qc == 0:
                        g1.append(emit_f1(1, of, psum_tile(f"pv{of % 2}")[:]))
                    w2prev = next_w2()
                    gg = T(tr, [128, QCH], F16, "gg", bufs=4)
                    nc.scalar.activation(gg[:], gsrc[of][:],
                                         AF.Gelu_apprx_tanh,
                                         bias=vap('b1', of)
                                         if flags['b1'] else 0.0)
                    for of2 in range(JE):
                        nc.tensor.matmul(ypt[of2],
                                         w2prev[:, of2 * 128:(of2 + 1) * 128],
                                         gg[:],
                                         start=(of == 0), stop=(of == JF - 1))
                for of2 in range(JE):
                    yo = T(tr, [128, QCH], F32, "yout", bufs=3)
                    if flags['b2']:
                        nc.vector.tensor_scalar(yo[:], ypt[of2], vap('b2', of2),
                                                None, op0=OP.add)
                    elif of2 % 2 == 0:
                        nc.vector.tensor_copy(yo[:], ypt[of2])
                    else:
                        nc.scalar.copy(yo[:], ypt[of2])
                    ring = [nc.sync, nc.scalar, nc.gpsimd][of2 % 3]
                    ring.dma_start(yT_d[of2 * 128:(of2 + 1) * 128, qs], yo[:])

    nc.compile()
    return nc


def kernel(**inputs):
    import ml_dtypes
    F8NP = ml_dtypes.float8_e4m3

    inp = {k: np.asarray(v, dtype=np.float32) for k, v in inputs.items()}
    triv1 = lambda v: bool(np.all(v == 1.0))
    triv0 = lambda v: bool(np.all(v == 0.0))
    flags = {
        'cn': not (triv1(inp['cn_g']) and triv0(inp['cn_b'])),
        'sn': not (triv1(inp['sn_g']) and triv0(inp['sn_b'])),
        'fn': not (triv1(inp['fn_g']) and triv0(inp['fn_b'])),
        'bq_c': not triv0(inp['bq_c']), 'bk_c': not triv0(inp['bk_c']),
        'bo_c': not triv0(inp['bo_c']), 'bq_s': not triv0(inp['bq_s']),
        'bk_s': not triv0(inp['bk_s']), 'bo_s': not triv0(inp['bo_s']),
        'b1': not triv0(inp['b1']), 'b2': not triv0(inp['b2']),
    }
    assert triv0(inp['bv_c']) and triv0(inp['bv_s']), \
        "nonzero V bias not supported by this build"

    key = tuple(sorted(flags.items()))
    if key not in _BUILT:
        _BUILT[key] = _build(flags)
    nc = _BUILT[key]

    from concourse.bass_utils import run_bass_kernel_spmd

    f16 = np.float16
    scale = 1.0 / np.sqrt(np.float32(D))

    def q8(x):
        return np.clip(x * SW, -224.0, 224.0).astype(F8NP)

    def tile_w8(W, jin, ofn):
        return np.ascontiguousarray(q8(
            W.reshape(jin, 128, ofn, 128).transpose(2, 1, 0, 3)
            .reshape(ofn, 128, jin * 128)))

    def tile_w16(W, jin, ofn):
        return np.ascontiguousarray(
            W.reshape(jin, 128, ofn, 128).transpose(2, 1, 0, 3)
            .reshape(ofn, 128, jin * 128).astype(f16))

    def tile_v8(W, jin):
        return np.ascontiguousarray(q8(W.reshape(jin, 128, E)))

    com = {
        'wq_c': tile_w8(inp['wq_c'] * scale, JE, JE),
        'wk_c': tile_w8(inp['wk_c'], JC, JE),
        'wv_c': tile_v8(inp['wv_c'], JC),
        'wo_c': tile_w8(inp['wo_c'], JE, JE),
        'wq_s': tile_w8(inp['wq_s'] * scale, JE, JE),
        'wk_s': tile_w8(inp['wk_s'], JE, JE),
        'wv_s': tile_v8(inp['wv_s'], JE),
        'wo_s': tile_w8(inp['wo_s'], JE, JE),
        'w1': tile_w16(inp['w1'], JE, JF),
        'w2': np.ascontiguousarray(inp['w2'].reshape(JF, 128, E).astype(f16)),
        # pe = exp(s)*eb carries x16 so fp8 keeps mantissa for small probs
        'expb_c': np.ascontiguousarray(
            (SX * np.exp(inp['bias_c'].transpose(0, 2, 1))).astype(f16)),
        'expb_s': np.ascontiguousarray(
            (SX * np.exp(inp['bias_s'].transpose(0, 2, 1))).astype(f16)),
    }
    chunks = []
    # LN betas ride the x16 activation scale in fp8 phases (cn, sn)
    for nm, s in [('cn_g', 1.0), ('cn_b', SX), ('sn_g', 1.0), ('sn_b', SX),
                  ('fn_g', 1.0), ('fn_b', 1.0)]:
        chunks.append((inp[nm] * s).reshape(-1, 128))
    chunks.append((inp['bq_c'] * scale).reshape(-1, 128))
    for nm in ['bk_c', 'bo_c']:
        chunks.append(inp[nm].reshape(-1, 128))
    chunks.append((inp['bq_s'] * scale).reshape(-1, 128))
    for nm in ['bk_s', 'bo_s', 'b1', 'b2']:
        chunks.append(inp[nm].reshape(-1, 128))
    com['vecs'] = np.ascontiguousarray(np.concatenate(chunks, 0).T)

    in_maps = []
    for b in range(B):
        m = dict(com)
        m['xT'] = np.ascontiguousarray(inp['hidden_state'][b].T)
        m['ctxT'] = np.ascontiguousarray(
            np.clip(inp['context'][b].T * SX, -224.0, 224.0).astype(F8NP))
        in_maps.append(m)

    res = run_bass_kernel_spmd(nc, in_maps, core_ids=list(range(NCORES)),
                               trace=TRACE)
    LAST['res'] = res
    y = np.stack([res.results[c]['yT'].T for c in range(B)])
    return np.ascontiguousarray(y.astype(np.float32))


# revision 30
# speedup vs baseline: 1.0348x; 1.0348x over previous
"""Trainium2 Bass kernel for nn_BasicTransformerLayer (dense transformer layer).

Strategy:
- Data-parallel over batch: B=8, one batch element per NeuronCore, no
  collectives (compute-bound regime; per-core work is identical).
- Activations transposed [features, tokens]; residual stream in fp32r.
- fp8e4m3 DoubleRow matmuls (2 contraction k-tiles per MM) for every
  attention-path projection (Q/K/V/O, both attns) and for P@V; the FFN
  stays f16 (fp8 there costs ~5e-2 rel err — it feeds the output with no
  residual damping). Scales (all powers of 2): activations x16, weights
  x256; descale 2^-12 folds into the psum-read op. The V tiles carry x16
  so AT=PV/sums lands pre-scaled for the fp8 O-proj with no extra op.
- Pair-tile layouts for DoubleRow 3D APs: LN outputs, context, V and AT
  live as [128, 2*W] tiles (k-tile pairs side by side in the free dim).
- Softmax: scores for pairs of kv tiles land in one [128,1024] 2-bank psum
  so exp amortizes the scalar engine's ACTIVATE overhead; the
  multiplicative exp(bias)*16 (host-precomputed f16) applies on vector in
  f16 (2x DVE) casting to the fp8 pe used by the DoubleRow P@V; the kv-sum
  (softmax denominator) is folded into the P@V matmul via V tiles laid out
  [64 ones | 64 values] per head: psum rows 0:64 = sums, 64:128 = P@V.
- PSUM: 3 double-bank score tiles + 2 P@V banks; BLK=3 pair-blocks keep the
  score-psum WAR distance >= the exp latency so the PE never burst-stalls.
- Q-projections run as fillers inside the previous attention loop's idle
  slots; fn-layernorm runs as post_qc of self-attn qc0 to overlap qc1.
- exp(bias) streams on the gpsimd DMA ring only; xT splits across the
  scalar+vector rings so LN1 starts ~7us in; output written per-psum-bank
  with rotating buffers over 3 DMA rings, copies alternating DVE/ACT.
"""
import sys

sys.path.insert(0, '/opt/trn_rl_repo')

import numpy as np

E, C, H, D, FF = 768, 512, 12, 64, 3072
B, S, L = 8, 1024, 256
EPS = 1e-5
NCORES = 8
QCH = 512                  # q-chunk (matmul moving free dim)
NQ = S // QCH              # 2
JE = E // 128              # 6 feature tiles
JC = C // 128              # 4 cross-feature tiles
JF = FF // 128             # 24 ffn tiles
KVS = S // 128             # 8 self kv tiles
KVC = L // 128             # 2 cross kv tiles

SX = 16.0                  # fp8 activation scale
SW = 256.0                 # fp8 weight scale
DS = 1.0 / (SX * SW)       # psum descale 2^-12

_BUILT = {}
TRACE = False
LAST = {}
PHASES = []


def _build(flags):
    import concourse.bacc as bacc
    import concourse.mybir as mybir
    import concourse.tile as tile
    from concourse.tile import add_dep_helper
    from concourse.bass import AP as BassAP

    FR = mybir.dt.float32r
    F32 = mybir.dt.float32
    F16 = mybir.dt.float16
    F8 = mybir.dt.float8e4
    AF = mybir.ActivationFunctionType
    OP = mybir.AluOpType
    DRM = mybir.MatmulPerfMode.DoubleRow

    nc = bacc.Bacc("TRN2", target_bir_lowering=False, debug=False,
                   enable_asserts=True, num_devices=NCORES)

    def din(name, shape, dt=F16):
        return nc.dram_tensor(name, shape, dt, kind="ExternalInput").ap()

    xT_d = din("xT", [E, S], F32)
    ctxT_d = din("ctxT", [C, L], F8)
    # pre-tiled fp8 weights: [ofn, 128, jin*128] (contiguous per of)
    w_d = {
        'wq_c': din("wq_c", [JE, 128, JE * 128], F8),
        'wk_c': din("wk_c", [JE, 128, JC * 128], F8),
        'wo_c': din("wo_c", [JE, 128, JE * 128], F8),
        'wq_s': din("wq_s", [JE, 128, JE * 128], F8),
        'wk_s': din("wk_s", [JE, 128, JE * 128], F8),
        'wo_s': din("wo_s", [JE, 128, JE * 128], F8),
        'w1': din("w1", [JF, 128, JE * 128]),
        'w2': din("w2", [JF, 128, JE * 128]),
        # V weights: [jin, 128, E] (row blocks of original)
        'wv_c': din("wv_c", [JC, 128, E], F8),
        'wv_s': din("wv_s", [JE, 128, E], F8),
    }
    expb_c_d = din("expb_c", [H, L, S])
    expb_s_d = din("expb_s", [H, S, S])
    VIDX = {}
    _off = 0
    for nm, ln in [('cn_g', JE), ('cn_b', JE), ('sn_g', JE), ('sn_b', JE),
                   ('fn_g', JE), ('fn_b', JE), ('bq_c', JE), ('bk_c', JE),
                   ('bo_c', JE), ('bq_s', JE), ('bk_s', JE), ('bo_s', JE),
                   ('b1', JF), ('b2', JE)]:
        VIDX[nm] = _off
        _off += ln
    NV = _off
    vecs_d = din("vecs", [128, NV], F32)
    yT_d = nc.dram_tensor("yT", [E, S], F32, kind="ExternalOutput").ap()

    with tile.TileContext(nc) as tc:
        with tc.tile_pool(name="const", bufs=1) as cpool, \
             tc.tile_pool(name="acts", bufs=1) as acts, \
             tc.tile_pool(name="wst", bufs=8) as wst, \
             tc.tile_pool(name="tr", bufs=2) as tr, \
             tc.tile_pool(name="pe", bufs=4) as pepool, \
             tc.tile_pool(name="eb", bufs=4) as ebpool, \
             tc.tile_pool(name="ps", bufs=1, space="PSUM") as ps:

            def T(pool, shape, dtype, tag, bufs=1):
                return pool.tile(shape, dtype, tag=tag, name=tag, bufs=bufs)

            ones = T(cpool, [128, 128], FR, "ones")
            ones16 = T(cpool, [128, 128], F16, "ones16")
            ones_f = T(cpool, [128, 128], F32, "ones_f")
            epsc = T(cpool, [128, 1], F32, "epsc")
            epsc8 = T(cpool, [128, 1], F32, "epsc8")
            nc.vector.memset(epsc[:], EPS)
            nc.vector.memset(epsc8[:], EPS / (SX * SX))
            nc.vector.memset(ones_f[:], 1.0)
            nc.vector.tensor_copy(ones[:], ones_f[:])
            nc.vector.tensor_copy(ones16[:], ones_f[:])
            vecs = T(cpool, [128, NV], F32, "vecs")
            # gpsimd ring: its first eb load is far away, so vecs lands early
            nc.gpsimd.dma_start(vecs[:], vecs_d[:])

            def vap(nm, j):
                return vecs[:, VIDX[nm] + j:VIDX[nm] + j + 1]

            # persistent activation tiles
            rA = [T(acts, [128, S], FR, f"rA{j}") for j in range(JE)]
            rB = [T(acts, [128, S], FR, f"rB{j}") for j in range(JE)]
            # fp8 LN outputs as pair tiles (k-tile pairs side by side)
            lnP = [T(acts, [128, 2 * S], F8, f"lnP{p}") for p in range(JE // 2)]
            # f16 LN output for the FFN path
            lnF = [T(acts, [128, S], F16, f"lnF{j}") for j in range(JE)]
            KT = [T(acts, [128, S], F16, f"KT{j}") for j in range(JE)]
            # V pair tiles: per kv-tile-pair [128, 2*H*128] fp8;
            # within each half, head h cols = [64 ones | 64 values]
            VP = [T(acts, [128, 2 * H * 128], F8, f"VP{p}")
                  for p in range(KVS // 2)]
            QT = [T(acts, [128, QCH], F16, f"QT{j}") for j in range(JE)]
            QTb = [T(acts, [128, QCH], F16, f"QU{j}") for j in range(JE)]
            # fp8 attention output as pair tiles
            AT8 = [T(acts, [128, 2 * QCH], F8, f"AT{p}") for p in range(JE // 2)]
            ctxP = [T(acts, [128, 2 * L], F8, f"cx{p}") for p in range(JC // 2)]

            def pair3(tile_, two_stride, n, off=0):
                """3D AP [[p,128],[two_stride,2],[1,n]] at free offset off."""
                vp = tile_[:]
                return BassAP(vp.tensor, vp.offset + off,
                              [[vp.ap[0][0], 128], [two_stride, 2], [1, n]])

            def vp_strided(t, head0, nh, ones_cols):
                vp = VP[t // 2][:]
                pstride = vp.ap[0][0]
                off = (vp.offset + (t % 2) * H * 128 + head0 * 128
                       + (0 if ones_cols else 64))
                return BassAP(vp.tensor, off,
                              [[pstride, 128], [128, nh], [1, 64]])

            for t in range(KVS):
                nc.vector.memset(vp_strided(t, 0, H, True), 1.0)

            for p in range(JC // 2):
                nc.sync.dma_start(ctxP[p][:, 0:L],
                                  ctxT_d[(2 * p) * 128:(2 * p + 1) * 128, :])
                nc.sync.dma_start(ctxP[p][:, L:2 * L],
                                  ctxT_d[(2 * p + 1) * 128:(2 * p + 2) * 128, :])

            def load_xT():
                # scalar ring is idle at start: the whole first q-chunk
                # streams there in parallel with the cross-K/V weight DMAs
                # on the sync ring, so ln1 qc0 starts ~7us in
                for half in range(2):
                    cs_ = slice(half * QCH, (half + 1) * QCH)
                    for j in range(JE):
                        ring = nc.scalar if half == 0 else \
                            (nc.sync if j % 2 == 0 else nc.scalar)
                        ring.dma_start(
                            rA[j][:, cs_],
                            xT_d[j * 128:(j + 1) * 128, cs_].bitcast(FR))

            # PSUM: 3x double-bank "scp" tiles (scores/general) + 2 single
            # "pv" banks = 8 banks total.
            def psum_tile(tag, n=QCH):
                return ps.tile([128, n], F32, tag=tag, name=tag)

            def scp_tile(k):
                return psum_tile(f"scp{k % 3}", 2 * QCH)

            _rot = {'i': 0, 'cur': None}

            def rot_ps(n=QCH):
                i = _rot['i']
                _rot['i'] += 1
                if i % 2 == 0:
                    _rot['cur'] = scp_tile(i // 2)
                return _rot['cur'][:, (i % 2) * QCH:(i % 2) * QCH + n]

            # ---------------- layer norm (transposed layout) ----------------
            # fp8 mode: output = 16*(x-mu)*rstd into lnP pair tiles (the x16
            # rides in rstd via the ln-scale); f16 mode: plain into lnF.
            # rstd = exp(-0.5*ln((var+eps)/s)) keeps ACT on the exp table set
            # (no Sqrt table thrash); x^2 tiles ride on gpsimd.
            def ln_phase(src, gname, bname, affine, fp8, only_qc=None,
                         gps_apply=False):
                inv = 1.0 / float(E)
                for qc in range(NQ):
                    if only_qc is not None and qc != only_qc:
                        continue
                    qs = slice(qc * QCH, (qc + 1) * QCH)
                    sqs = []
                    for j in range(JE):
                        # ACT Square: in every table set, never thrashes
                        sq = T(tr, [128, QCH], F16, "sq", bufs=3)
                        nc.scalar.activation(sq[:], src[j][:, qs], AF.Square)
                        sqs.append(sq)
                    s12 = scp_tile(0)
                    s1 = s12[:, 0:QCH]
                    s2 = s12[:, QCH:2 * QCH]
                    for j in range(JE):
                        nc.tensor.matmul(s1, ones[:, 0:128], src[j][:, qs],
                                         start=(j == 0), stop=(j == JE - 1))
                    for j in range(JE):
                        nc.tensor.matmul(s2, ones16[:, 0:128], sqs[j][:],
                                         start=(j == 0), stop=(j == JE - 1))
                    t1 = T(tr, [128, QCH], F32, "t1m", bufs=1)
                    nc.scalar.activation(t1[:], s1[:], AF.Square, scale=inv)
                    var = T(tr, [128, QCH], F32, "var", bufs=1)
                    nc.vector.scalar_tensor_tensor(var[:], s2[:], inv, t1[:],
                                                   op0=OP.mult, op1=OP.subtract)
                    sc_ = (1.0 / (SX * SX)) if fp8 else 1.0
                    nc.scalar.activation(var[:], var[:], AF.Ln,
                                         bias=(epsc8 if fp8 else epsc)[:, 0:1],
                                         scale=sc_)
                    rstd = T(tr, [128, QCH], F32, "rstd", bufs=1)
                    nc.scalar.activation(rstd[:], var[:], AF.Exp, scale=-0.5)
                    m1r = T(tr, [128, QCH], F16, "m1r", bufs=1)
                    nc.vector.scalar_tensor_tensor(m1r[:], s1[:], inv, rstd[:],
                                                   op0=OP.mult, op1=OP.mult)
                    for j in range(JE):
                        if fp8:
                            dst = lnP[j // 2][:, (j % 2) * S + qc * QCH:
                                              (j % 2) * S + (qc + 1) * QCH]
                        else:
                            dst = lnF[j][:, qs]
                        tmp = T(tr, [128, QCH], F16, "lntmp", bufs=2)
                        nc.vector.tensor_tensor(tmp[:], src[j][:, qs], rstd[:],
                                                op=OP.mult)
                        if affine:
                            tmp2 = T(tr, [128, QCH], F16, "lntmp2", bufs=2)
                            nc.vector.tensor_tensor(tmp2[:], tmp[:], m1r[:],
                                                    op=OP.subtract)
                            nc.vector.tensor_scalar(dst, tmp2[:],
                                                    vap(gname, j), vap(bname, j),
                                                    op0=OP.mult, op1=OP.add)
                        else:
                            nc.vector.tensor_tensor(dst, tmp[:],
                                                    m1r[:], op=OP.subtract)

            # -------- fp8 DoubleRow projection from pre-tiled weights --------
            def wload(wd, of, jin, ring=None):
                wt = T(wst, [128, JE * 128], F8, "wg", bufs=4)
                (ring or nc.sync).dma_start(wt[:, 0:jin * 128], wd[of])
                return wt

            _cpn = {'i': 0}

            def psum_read(out_ap, pt, bias_ap):
                # out = pt * 2^-12 (+ bias); gpsimd can't read PSUM, so
                # rotate 2:1 vector:scalar (ACT carries the exp stream)
                if bias_ap is not None:
                    nc.vector.tensor_scalar(out_ap, pt, DS, bias_ap,
                                            op0=OP.mult, op1=OP.add)
                elif _cpn['i'] % 2 == 1:
                    _cpn['i'] += 1
                    nc.scalar.mul(out_ap, pt, DS)
                else:
                    _cpn['i'] += 1
                    nc.vector.tensor_scalar(out_ap, pt, DS, None, op0=OP.mult)

            def wchain(wt, jin, src3, out_ap, n, bias_ap, pt=None):
                """src3(p) -> 3D rhs AP for k-tile pair p."""
                if pt is None:
                    pt = rot_ps(n)
                wv_ = wt[:]
                for p in range(jin // 2):
                    w3 = BassAP(wv_.tensor, wv_.offset + p * 256,
                                [[wv_.ap[0][0], 128], [128, 2], [1, 128]])
                    nc.tensor.matmul(pt, w3, src3(p),
                                     start=(p == 0), stop=(p == jin // 2 - 1),
                                     perf_mode=DRM)
                psum_read(out_ap, pt, bias_ap)

            def ln_src3(qs0):
                return lambda p: pair3(lnP[p], S, QCH, off=qs0)

            def qproj_closures(prefix, wq, qs0, qt_set, pslot=None):
                outs = []
                for of in range(JE):
                    def one(of=of):
                        wt = wload(wq, of, JE)
                        wchain(wt, JE, ln_src3(qs0),
                               qt_set[of][:], QCH,
                               vap(f'bq_{prefix}', of)
                               if flags[f'bq_{prefix}'] else None,
                               pt=pslot() if pslot else None)
                    outs.append(one)
                return outs

            _frot = {'i': 0, 'cur': None}

            def fill_ps(n=QCH):
                i = _frot['i']
                _frot['i'] += 1
                if i % 2 == 0:
                    _frot['cur'] = psum_tile("scp2", 2 * QCH)
                return _frot['cur'][:, (i % 2) * QCH:(i % 2) * QCH + n]

            # ---------------- K/V projection emission ----------------
            def emit_kv(prefix, kv_pairs, kv_stride, wk, wv, jin_kv, kv_len):
                nkv = kv_len // 128

                _kps = {'i': 0}

                def emit_k(of):
                    wt = wload(wk, of, jin_kv)
                    for ks in range(0, kv_len, QCH):
                        n = min(QCH, kv_len - ks)
                        kp = psum_tile(f"pv{_kps['i'] % 2}")
                        _kps['i'] += 1
                        wchain(wt, jin_kv,
                               lambda p: pair3(kv_pairs[p], kv_stride, n,
                                               off=ks),
                               KT[of][:, ks:ks + n], n,
                               vap(f'bk_{prefix}', of)
                               if flags[f'bk_{prefix}'] else None,
                               pt=kp[:, 0:n])

                def emit_vgroup(os_, tg):
                    n = min(QCH, E - os_)
                    tcnt = min(4, nkv - tg)
                    vts = [scp_tile(1), scp_tile(2)]
                    vps = [vts[i // 2][:, (i % 2) * QCH:(i % 2) * QCH + n]
                           for i in range(tcnt)]
                    for jp in range(jin_kv // 2):
                        wt = T(wst, [128, 2 * QCH], F8, "wv", bufs=4)
                        nc.sync.dma_start(wt[:, 0:n],
                                          wv[2 * jp, :, os_:os_ + n])
                        nc.sync.dma_start(wt[:, n:2 * n],
                                          wv[2 * jp + 1, :, os_:os_ + n])
                        for i in range(tcnt):
                            nc.tensor.matmul(
                                vps[i][:, 0:n],
                                pair3(kv_pairs[jp], kv_stride, 128,
                                      off=(tg + i) * 128),
                                pair3(wt, n, n),
                                start=(jp == 0),
                                stop=(jp == jin_kv // 2 - 1),
                                perf_mode=DRM)
                    for i in range(tcnt):
                        dst = vp_strided(tg + i, os_ // 64, n // 64, False)
                        src = vps[i][:, 0:n].rearrange("p (h d) -> p h d", d=64)
                        # V carries x16: psum(4096) * 2^-8 = 16*V
                        nc.scalar.mul(dst, src, DS * SX)

                vgroups = [(os_, tg) for os_ in range(0, E, QCH)
                           for tg in range(0, nkv, 4)]
                for i in range(max(JE, len(vgroups))):
                    if i < JE:
                        emit_k(i)
                    if i < len(vgroups):
                        emit_vgroup(*vgroups[i])

            # ---------------- attention (shared cross/self) ----------------
            def attention(prefix, kv_pairs, kv_stride, expb_d, res_in,
                          res_out, wq, wk, wv, wo, jin_kv, kv_len,
                          post_qc=None, kv_done=False, qt_sets=None,
                          emit_q=(True, True), fillers=None, rot_mods=(3, 3)):
                if not kv_done:
                    emit_kv(prefix, kv_pairs, kv_stride, wk, wv, jin_kv,
                            kv_len)
                if qt_sets is None:
                    qt_sets = [QT, QT]

                for qc in range(NQ):
                    PHASES.append((f'{prefix}:qc{qc}',
                                   int(__import__('re').findall(
                                       r'\d+', nc.get_next_instruction_name())[0])))
                    rot_mod = rot_mods[qc]
                    qs = slice(qc * QCH, (qc + 1) * QCH)
                    qt = qt_sets[qc]
                    # Q^T for this q-chunk (scale folded into wq on host)
                    if emit_q[qc]:
                        for fn_ in qproj_closures(prefix, wq, qc * QCH, qt):
                            fn_()
                    fq = list(fillers[qc]) if fillers else []
                    nkt = kv_len // 128
                    npair = nkt // 2
                    seq = [(h, kp) for h in range(H) for kp in range(npair)]
                    _sr = {'i': 0}
                    state = {}

                    def load_eb(h):
                        ebts = []
                        for kp in range(npair):
                            ebt = T(ebpool, [128, 2 * QCH], F16, "eb", bufs=8)
                            # split the eb stream across two DMA rings
                            ring = nc.gpsimd if (h + kp) % 2 else nc.sync
                            ring.dma_start(
                                ebt[:].rearrange("p (t c) -> p t c", t=2),
                                expb_d[h, kp * 256:(kp + 1) * 256, qs]
                                .rearrange("(t p) c -> p t c", p=128))
                            ebts.append(ebt)
                        state.setdefault(h, {'tiles': []})['ebts'] = ebts

                    load_eb(0)

                    def s_stage(i):
                        h, kp = seq[i]
                        st = state.setdefault(h, {'tiles': []})
                        if kp == 0 and h + 1 < H:
                            load_eb(h + 1)
                        th, ph = (h * D) // 128, (h * D) % 128
                        sc = scp_tile(_sr['i'] % rot_mod)
                        _sr['i'] += 1
                        for half in range(2):
                            kvt = 2 * kp + half
                            chain(nc.tensor.matmul(
                                sc[:, half * QCH:(half + 1) * QCH],
                                KT[th][ph:ph + D, kvt * 128:(kvt + 1) * 128],
                                qt[th][ph:ph + D, :], start=True, stop=True))
                        pe = T(pepool, [128, 2 * QCH], F16, "pe", bufs=4)
                        pe8 = T(pepool, [128, 2 * QCH], F8, "pe8", bufs=6)
                        nc.scalar.activation(pe[:], sc[:], AF.Exp)
                        # 1 in 4 self exp(bias) multiplies rides on gpsimd;
                        # more than that taxes DVE via SBUF port contention
                        eng = nc.gpsimd if (npair > 1 and kp == 3) \
                            else nc.vector
                        eng.tensor_tensor(pe8[:], pe[:],
                                          st['ebts'][kp][:], op=OP.mult)
                        st['tiles'].append(pe8)

                    def pv_stage(i):
                        h, kp = seq[i]
                        st = state[h]
                        th, ph = (h * D) // 128, (h * D) % 128
                        if kp == 0:
                            st['pv'] = psum_tile(f"pv{h % 2}")
                        pe8 = st['tiles'][kp]
                        vp = VP[kp][:]
                        lhs3 = BassAP(vp.tensor, vp.offset + h * 128,
                                      [[vp.ap[0][0], 128], [H * 128, 2],
                                       [1, 128]])
                        chain(nc.tensor.matmul(
                            st['pv'][:], lhs3, pair3(pe8, QCH, QCH),
                            start=(kp == 0), stop=(kp == npair - 1),
                            perf_mode=DRM))
                        if kp == npair - 1:
                            pv = st['pv']

                            def fin(pv=pv, th=th, ph=ph, h=h):
                                # psum rows 0:64 = kv-sums, 64:128 = P@V(x16)
                                rec = T(tr, [64, QCH], F32, "rec", bufs=2)
                                nc.vector.reciprocal_approx_fast(
                                    rec[:], pv[0:64, :])
                                dst = AT8[th // 2][ph:ph + D,
                                                  (th % 2) * QCH:
                                                  (th % 2 + 1) * QCH]
                                nc.vector.tensor_tensor(dst, pv[64:128, :],
                                                        rec[:], op=OP.mult)
                            deferred.append(fin)
                            del state[h]

                    _pe_chain = {'prev': None}

                    def chain(bi):
                        if _pe_chain['prev'] is not None:
                            add_dep_helper(bi.ins, _pe_chain['prev'].ins,
                                           sync=False, reason="pe-order")
                        _pe_chain['prev'] = bi

                    deferred = []
                    BLK = 3
                    blocks = [list(range(i, min(i + BLK, len(seq))))
                              for i in range(0, len(seq), BLK)]
                    for j in range(len(blocks) + 1):
                        if j < len(blocks):
                            for i in blocks[j]:
                                s_stage(i)
                        if fq and j >= 1:
                            # pace the backlog evenly over remaining blocks
                            npop = -(-len(fq) // max(1, len(blocks) + 1 - j))
                            for _ in range(min(npop, len(fq))):
                                fq.pop(0)()
                        while len(deferred) > 1:
                            deferred.pop(0)()
                        if j >= 1:
                            for i in blocks[j - 1]:
                                pv_stage(i)
                    while deferred:
                        deferred.pop(0)()
                    for fn_ in fq:
                        fn_()
                    # out-projection + residual (AT carries x16, wo x256)
                    at3 = lambda p: pair3(AT8[p], QCH, QCH)
                    for of in range(JE):
                        wt = wload(wo, of, JE)
                        pt = rot_ps()
                        wv_ = wt[:]
                        for p in range(JE // 2):
                            w3 = BassAP(wv_.tensor, wv_.offset + p * 256,
                                        [[wv_.ap[0][0], 128], [128, 2],
                                         [1, 128]])
                            nc.tensor.matmul(pt, w3, at3(p),
                                             start=(p == 0),
                                             stop=(p == JE // 2 - 1),
                                             perf_mode=DRM)
                        if flags[f'bo_{prefix}']:
                            tmp = T(tr, [128, QCH], F32, "obias", bufs=2)
                            nc.vector.tensor_scalar(tmp[:], pt, DS,
                                                    vap(f'bo_{prefix}', of),
                                                    op0=OP.mult, op1=OP.add)
                            nc.vector.tensor_tensor(res_out[of][:, qs], tmp[:],
                                                    res_in[of][:, qs],
                                                    op=OP.add)
                        else:
                            nc.vector.scalar_tensor_tensor(
                                res_out[of][:, qs], pt, DS,
                                res_in[of][:, qs], op0=OP.mult, op1=OP.add)
                    if post_qc is not None:
                        post_qc(qc)

            # ================= the layer =================
            import re as _re

            def _mark(lbl):
                n = int(_re.findall(r'\d+', nc.get_next_instruction_name())[0])
                PHASES.append((lbl, n))

            _mark('ckv')
            # cross K/V only need ctxP: emit first to overlap xT DMA
            emit_kv('c', ctxP, L, w_d['wk_c'], w_d['wv_c'], JC, L)
            load_xT()
            _mark('ln1')
            ln_phase(rA, 'cn_g', 'cn_b', flags['cn'], fp8=True)
            _mark('cross')
            fill_q0 = qproj_closures('c', w_d['wq_c'], QCH, QTb,
                                     pslot=fill_ps)
            fill_q1 = qproj_closures('s', w_d['wq_s'], 0, QT,
                                     pslot=fill_ps)
            attention('c', ctxP, L, expb_c_d, rA, rB,
                      w_d['wq_c'], w_d['wk_c'], w_d['wv_c'], w_d['wo_c'],
                      JC, L, kv_done=True, rot_mods=(2, 2),
                      qt_sets=[QT, QTb], emit_q=(True, False),
                      fillers=[fill_q0, fill_q1],
                      post_qc=lambda qc: ln_phase(rB, 'sn_g', 'sn_b',
                                                  flags['sn'], fp8=True,
                                                  only_qc=qc,
                                                  gps_apply=True))
            # ---- FFN first-matmul emission (shared by filler + main) ----
            # Writes pre-gelu f16 tiles; the gelu itself batches into the
            # FFN phases (Exp and Gelu can NEVER share an ACT table set, so
            # a gelu inside the attention window would thrash tables; the
            # psum-read here is a Copy, which lives in every set).
            def emit_f1(qc, of, f1_ap):
                qs_ = slice(qc * QCH, (qc + 1) * QCH)
                wt = T(wst, [128, JE * 128], F16, "w1g", bufs=3)
                nc.sync.dma_start(wt[:], w_d['w1'][of])
                for j in range(JE):
                    nc.tensor.matmul(f1_ap, wt[:, j * 128:(j + 1) * 128],
                                     lnF[j][:, qs_],
                                     start=(j == 0), stop=(j == JE - 1),
                                     skip_group_check=True)
                g = T(acts, [128, QCH], F16, "g", bufs=26)
                nc.scalar.copy(g[:], f1_ap)
                return g

            g0 = []
            _ft = {}

            def f1_filler_closures():
                # f1(qc0) of-tiles run as fillers inside self-attn qc1,
                # double-buffered in the two halves of ONE scp2 allocation
                # (lazy: scores in qc0 still rotate through scp2)
                outs = []
                for of in range(JF):
                    def one(of=of):
                        if 'ft' not in _ft:
                            _ft['ft'] = scp_tile(2)
                        f1 = _ft['ft']
                        g0.append(emit_f1(0, of,
                                          f1[:, (of % 2) * QCH:
                                             (of % 2 + 1) * QCH]))
                    outs.append(one)
                return outs

            _mark('self')
            # fn-ln per qc as post_qc: qc0's fn-ln + f1(qc0) overlap self qc1
            attention('s', lnP, S, expb_s_d, rB, rA,
                      w_d['wq_s'], w_d['wk_s'], w_d['wv_s'], w_d['wo_s'],
                      JE, S, qt_sets=[QT, QTb], emit_q=(False, True),
                      rot_mods=(3, 2),
                      fillers=[[], f1_filler_closures()],
                      post_qc=lambda qc: ln_phase(rA, 'fn_g', 'fn_b',
                                                  flags['fn'], fp8=False,
                                                  only_qc=qc))
            _mark('ffn')

            # ================= FFN second matmuls (f16) =================
            # phase B: w2 chain for qc0 (g0 precomputed) interleaved with
            # f1(qc1); phase C: w2 chain for qc1.
            def load_w2(of):
                w2t = T(wst, [128, JE * 128], F16, "w2g", bufs=3)
                nc.gpsimd.dma_start(w2t[:], w_d['w2'][of])
                return w2t

            g1 = []
            # w2 weight prefetch runs as one continuous stream across both
            # qc phases so the B->C transition never starves LDWEIGHTS
            w2q = [load_w2(0), load_w2(1)]
            _w2n = {'i': 2}

            def next_w2():
                if _w2n['i'] < 2 * JF:
                    w2q.append(load_w2(_w2n['i'] % JF))
                    _w2n['i'] += 1
                return w2q.pop(0)

            for qc in range(NQ):
                qs = slice(qc * QCH, (qc + 1) * QCH)
                ytiles = [scp_tile(k) for k in range(3)]
                ypt = [ytiles[k // 2][:, (k % 2) * QCH:(k % 2 + 1) * QCH]
                       for k in range(JE)]
                gsrc = g0 if qc == 0 else g1
                for of in range(JF):
                    if qc == 0:
                        g1.append(emit_f1(1, of, psum_tile(f"pv{of % 2}")[:]))
                    w2prev = next_w2()
                    gg = T(tr, [128, QCH], F16, "gg", bufs=4)
                    nc.scalar.activation(gg[:], gsrc[of][:],
                                         AF.Gelu_apprx_tanh,
                                         bias=vap('b1', of)
                                         if flags['b1'] else 0.0)
                    for of2 in range(JE):
                        nc.tensor.matmul(ypt[of2],
                                         w2prev[:, of2 * 128:(of2 + 1) * 128],
                                         gg[:],
                                         start=(of == 0), stop=(of == JF - 1))
                for of2 in range(JE):
                    yo = T(tr, [128, QCH], F32, "yout", bufs=3)
                    if flags['b2']:
                        nc.vector.tensor_scalar(yo[:], ypt[of2], vap('b2', of2),
                                                None, op0=OP.add)
                    elif of2 % 2 == 0:
                        nc.vector.tensor_copy(yo[:], ypt[of2])
                    else:
                        nc.scalar.copy(yo[:], ypt[of2])
                    ring = [nc.sync, nc.scalar, nc.gpsimd][of2 % 3]
                    ring.dma_start(yT_d[of2 * 128:(of2 + 1) * 128, qs], yo[:])

    nc.compile()
    return nc


def kernel(**inputs):
    import ml_dtypes
    F8NP = ml_dtypes.float8_e4m3

    inp = {k: np.asarray(v, dtype=np.float32) for k, v in inputs.items()}
    triv1 = lambda v: bool(np.all(v == 1.0))
    triv0 = lambda v: bool(np.all(v == 0.0))
    flags = {
        'cn': not (triv1(inp['cn_g']) and triv0(inp['cn_b'])),
        'sn': not (triv1(inp['sn_g']) and triv0(inp['sn_b'])),
        'fn': not (triv1(inp['fn_g']) and triv0(inp['fn_b'])),
        'bq_c': not triv0(inp['bq_c']), 'bk_c': not triv0(inp['bk_c']),
        'bo_c': not triv0(inp['bo_c']), 'bq_s': not triv0(inp['bq_s']),
        'bk_s': not triv0(inp['bk_s']), 'bo_s': not triv0(inp['bo_s']),
        'b1': not triv0(inp['b1']), 'b2': not triv0(inp['b2']),
    }
    assert triv0(inp['bv_c']) and triv0(inp['bv_s']), \
        "nonzero V bias not supported by this build"

    key = tuple(sorted(flags.items()))
    if key not in _BUILT:
        _BUILT[key] = _build(flags)
    nc = _BUILT[key]

    from concourse.bass_utils import run_bass_kernel_spmd

    f16 = np.float16
    scale = 1.0 / np.sqrt(np.float32(D))

    def q8(x):
        return np.clip(x * SW, -224.0, 224.0).astype(F8NP)

    def tile_w8(W, jin, ofn):
        return np.ascontiguousarray(q8(
            W.reshape(jin, 128, ofn, 128).transpose(2, 1, 0, 3)
            .reshape(ofn, 128, jin * 128)))

    def tile_w16(W, jin, ofn):
        return np.ascontiguousarray(
            W.reshape(jin, 128, ofn, 128).transpose(2, 1, 0, 3)
            .reshape(ofn, 128, jin * 128).astype(f16))

    def tile_v8(W, jin):
        return np.ascontiguousarray(q8(W.reshape(jin, 128, E)))

    com = {
        'wq_c': tile_w8(inp['wq_c'] * scale, JE, JE),
        'wk_c': tile_w8(inp['wk_c'], JC, JE),
        'wv_c': tile_v8(inp['wv_c'], JC),
        'wo_c': tile_w8(inp['wo_c'], JE, JE),
        'wq_s': tile_w8(inp['wq_s'] * scale, JE, JE),
        'wk_s': tile_w8(inp['wk_s'], JE, JE),
        'wv_s': tile_v8(inp['wv_s'], JE),
        'wo_s': tile_w8(inp['wo_s'], JE, JE),
        'w1': tile_w16(inp['w1'], JE, JF),
        'w2': np.ascontiguousarray(inp['w2'].reshape(JF, 128, E).astype(f16)),
        # pe = exp(s)*eb carries x16 so fp8 keeps mantissa for small probs
        'expb_c': np.ascontiguousarray(
            (SX * np.exp(inp['bias_c'].transpose(0, 2, 1))).astype(f16)),
        'expb_s': np.ascontiguousarray(
            (SX * np.exp(inp['bias_s'].transpose(0, 2, 1))).astype(f16)),
    }
    chunks = []
    # LN betas ride the x16 activation scale in fp8 phases (cn, sn)
    for nm, s in [('cn_g', 1.0), ('cn_b', SX), ('sn_g', 1.0), ('sn_b', SX),
                  ('fn_g', 1.0), ('fn_b', 1.0)]:
        chunks.append((inp[nm] * s).reshape(-1, 128))
    chunks.append((inp['bq_c'] * scale).reshape(-1, 128))
    for nm in ['bk_c', 'bo_c']:
        chunks.append(inp[nm].reshape(-1, 128))
    chunks.append((inp['bq_s'] * scale).reshape(-1, 128))
    for nm in ['bk_s', 'bo_s', 'b1', 'b2']:
        chunks.append(inp[nm].reshape(-1, 128))
    com['vecs'] = np.ascontiguousarray(np.concatenate(chunks, 0).T)

    in_maps = []
    for b in range(B):
        m = dict(com)
        m['xT'] = np.ascontiguousarray(inp['hidden_state'][b].T)
        m['ctxT'] = np.ascontiguousarray(
            np.clip(inp['context'][b].T * SX, -224.0, 224.0).astype(F8NP))
        in_maps.append(m)

    res = run_bass_kernel_spmd(nc, in_maps, core_ids=list(range(NCORES)),
                               trace=TRACE)
    LAST['res'] = res
    y = np.stack([res.results[c]['yT'].T for c in range(B)])
    return np.ascontiguousarray(y.astype(np.float32))
